# revision 20
# baseline (speedup 1.0000x reference)
"""Trainium2 Bass kernel for a fused CustomLSTMCell.

Math (reference):
    combined = concat([input, hidden], axis=1)            # [B, D], D = 2048
    gates    = combined @ concat([W_i,W_f,W_o,W_g], 1) + b  # [B, 4H]
    i, f, o, g = split(gates, 4, axis=1)
    new_cell   = sigmoid(f) * cell_state + sigmoid(i) * tanh(g)
    new_hidden = sigmoid(o) * tanh(new_cell)

Strategy:
  - Data-parallel over batch: 8 cores x 1024 rows each. No collectives.
  - Transposed W-stationary layout: gate columns (H) live on PSUM partitions,
    batch on the free dim. gates^T = W^T @ combined^T per 128-column h-block.
    Host prepares combined^T (bf16), W packed per h-block, cell_state^T.
  - The per-gate bias is then per-partition, so it folds into the ACT
    activation (out = sigmoid/tanh(psum + bias)) -- no DVE bias adds.
  - The cell/hidden elementwise chain runs on DVE in bf16 (2x rate). fp32
    drain math measured ~250us of DVE occupancy and periodically stalled the
    PE by delaying PSUM recycling; bf16 + ACT-bias cuts it to ~30us.
  - W streams as 8 x 2MB h-blocks through a 4-deep buffer ring, per-k DMA
    slices interleaved with the combined^T loads, so the PE never starves
    and the For_i back-edge overlaps.
"""

import sys

if "/opt/trn_rl_repo" not in sys.path:
    sys.path.insert(0, "/opt/trn_rl_repo")

import ml_dtypes
import numpy as np

import concourse.bass as bass
import concourse.mybir as mybir
import concourse.tile as tile
from concourse import bacc
from concourse.bass_utils import run_bass_kernel_spmd

N_CORES = 8
B = 8192
IN_SIZE = 1024
H = 1024
D = IN_SIZE + H          # 2048 contraction dim
G4 = 4 * H               # 4096 gate columns
BC = B // N_CORES        # 1024 batch rows per core
P = 128                  # partitions
KT = D // P              # 16 k-tiles
JT = H // P              # 8 h-blocks
NB = 512                 # batch columns per matmul (moving free dim)
BBLK = BC // NB          # 2 batch blocks

_NC_CACHE = {}


def _build(iters: int = 1) -> bass.Bass:
    nc = bacc.Bacc("TRN2", target_bir_lowering=False, debug=False)

    at = nc.dram_tensor("at", [D, BC], mybir.dt.bfloat16, kind="ExternalInput")
    wj = nc.dram_tensor("wj", [JT, P, KT * 512], mybir.dt.bfloat16, kind="ExternalInput")
    bv = nc.dram_tensor("bv", [P, 4 * JT], mybir.dt.float32, kind="ExternalInput")
    cst = nc.dram_tensor("cst", [JT, P, BC], mybir.dt.bfloat16, kind="ExternalInput")
    nht = nc.dram_tensor("nht", [JT, P, BC], mybir.dt.bfloat16, kind="ExternalOutput")
    nclt = nc.dram_tensor("nclt", [JT, P, BC], mybir.dt.bfloat16, kind="ExternalOutput")

    at_r = at.rearrange("(ko ki) b -> ki ko b", ki=P)   # [128, KT, BC]

    AF = mybir.ActivationFunctionType

    from contextlib import nullcontext

    with tile.TileContext(nc) as tc:
        with (
            tc.tile_pool(name="resident", bufs=1) as rpool,
            tc.tile_pool(name="wstream", bufs=4) as wspool,
            tc.tile_pool(name="work", bufs=2) as wpool,
            tc.tile_pool(name="psum", bufs=2, space="PSUM") as ppool,
        ):
            with (tc.For_i(0, iters, 1) if iters > 1 else nullcontext()):
                bv_sb = rpool.tile([P, 4 * JT], mybir.dt.float32, tag="bv")
                nc.sync.dma_start(out=bv_sb[:], in_=bv[:])
                at_sb = rpool.tile([P, KT, BC], mybir.dt.bfloat16, tag="at")
                for j in range(JT):
                    # stream this h-block of W (2 MB), interleaved per-k with
                    # the combined^T loads on the first block
                    wj_sb = wspool.tile([P, KT, 512], mybir.dt.bfloat16, tag="wj", name="wj_sb")
                    for k in range(KT):
                        nc.sync.dma_start(
                            out=wj_sb[:, k, :], in_=wj[j, :, k * 512 : (k + 1) * 512]
                        )
                        if j == 0:
                            nc.sync.dma_start(out=at_sb[:, k, :], in_=at_r[:, k, :])
                    for b in range(BBLK):
                        # prefetch the cell-state block a full group early
                        cl = wpool.tile([P, NB], mybir.dt.bfloat16, tag="cl")
                        nc.sync.dma_start(
                            out=cl[:], in_=cst[j, :, b * NB : (b + 1) * NB]
                        )
                        ps = [
                            ppool.tile([P, NB], mybir.dt.float32, tag=f"ps{q}", name=f"ps{q}")
                            for q in range(4)
                        ]
                        for k in range(KT):
                            mov = at_sb[:, k, b * NB : (b + 1) * NB]
                            for q in range(4):
                                nc.tensor.matmul(
                                    ps[q][:],
                                    lhsT=wj_sb[:, k, q * P : (q + 1) * P],
                                    rhs=mov,
                                    start=(k == 0),
                                    stop=(k == KT - 1),
                                )
                        # gate activations with fused per-partition bias,
                        # PSUM -> SBUF bf16
                        gt = [
                            wpool.tile([P, NB], mybir.dt.bfloat16, tag=f"gt{q}", name=f"gt{q}")
                            for q in range(4)
                        ]
                        for q in range(4):
                            nc.scalar.activation(
                                gt[q][:],
                                ps[q][:],
                                AF.Tanh if q == 3 else AF.Sigmoid,
                                bias=bv_sb[:, q * JT + j : q * JT + j + 1],
                            )
                        si, sf, so, sg = gt
                        # new_cell = sigmoid(f)*c + sigmoid(i)*tanh(g)  -> sf
                        nc.vector.tensor_mul(out=sf[:], in0=sf[:], in1=cl[:])
                        nc.vector.tensor_mul(out=si[:], in0=si[:], in1=sg[:])
                        nc.vector.tensor_add(out=sf[:], in0=sf[:], in1=si[:])
                        # new_hidden = sigmoid(o)*tanh(new_cell)        -> so
                        nc.scalar.activation(sg[:], sf[:], AF.Tanh)
                        nc.vector.tensor_mul(out=so[:], in0=so[:], in1=sg[:])
                        nc.sync.dma_start(
                            out=nclt[j, :, b * NB : (b + 1) * NB], in_=sf[:]
                        )
                        nc.sync.dma_start(
                            out=nht[j, :, b * NB : (b + 1) * NB], in_=so[:]
                        )
    nc.finalize()
    return nc


def get_nc(iters: int = 1) -> bass.Bass:
    if iters not in _NC_CACHE:
        _NC_CACHE[iters] = _build(iters)
    return _NC_CACHE[iters]


def make_in_maps(input, hidden, cell_state, W_i, b_i, W_f, b_f, W_o, b_o, W_g, b_g):
    comb = np.concatenate(
        [np.asarray(input, np.float32), np.asarray(hidden, np.float32)], axis=1
    )  # [B, D]
    W4 = np.concatenate(
        [np.asarray(W_i), np.asarray(W_f), np.asarray(W_o), np.asarray(W_g)], axis=1
    ).astype(np.float32)  # [D, 4H]
    b = np.concatenate(
        [np.asarray(b_i), np.asarray(b_f), np.asarray(b_o), np.asarray(b_g)]
    ).astype(np.float32)  # [4H]

    at_full = comb.T.astype(ml_dtypes.bfloat16)  # [D, B]
    # W packed per h-block j: [128 ki, KT, (q, col)] so per-k DMA slices are
    # contiguous and the stationary operand for (k, q) is wj[:, k, q*128:...]
    wj = np.empty((JT, P, KT * 512), dtype=ml_dtypes.bfloat16)
    for j in range(JT):
        blk = np.concatenate(
            [W4[:, q * H + j * P : q * H + (j + 1) * P] for q in range(4)], axis=1
        )  # [D, 512], col = q*128 + c
        wj[j] = (
            blk.astype(ml_dtypes.bfloat16)
            .reshape(KT, P, 512)
            .transpose(1, 0, 2)
            .reshape(P, KT * 512)
        )
    bv = np.ascontiguousarray(
        b.reshape(4, JT, P).transpose(2, 0, 1).reshape(P, 4 * JT)
    )
    cs = np.asarray(cell_state, np.float32)

    in_maps = []
    for c in range(N_CORES):
        sl = slice(c * BC, (c + 1) * BC)
        cst = np.ascontiguousarray(
            cs[sl].T.reshape(JT, P, BC).astype(ml_dtypes.bfloat16)
        )
        in_maps.append(
            {
                "at": np.ascontiguousarray(at_full[:, sl]),
                "wj": wj,
                "bv": bv,
                "cst": cst,
            }
        )
    return in_maps


def kernel(input, hidden, cell_state, W_i, b_i, W_f, b_f, W_o, b_o, W_g, b_g):
    in_maps = make_in_maps(
        input, hidden, cell_state, W_i, b_i, W_f, b_f, W_o, b_o, W_g, b_g
    )
    nc = get_nc(1)

    def run_once():
        res = run_bass_kernel_spmd(nc, in_maps, core_ids=list(range(N_CORES)))
        return (
            [np.asarray(res.results[c]["nht"]).copy() for c in range(N_CORES)],
            [np.asarray(res.results[c]["nclt"]).copy() for c in range(N_CORES)],
        )

    # The device very occasionally corrupts an execution (observed ~once in
    # 14 runs on this host). The kernel is deterministic, so run twice and
    # accept only agreeing outputs; retry once more on mismatch.
    prev = run_once()
    for _ in range(2):
        cur = run_once()
        if all(
            np.array_equal(a, b) for a, b in zip(prev[0] + prev[1], cur[0] + cur[1])
        ):
            break
        prev = cur
    nht_res, nclt_res = prev
    new_hidden = np.concatenate(
        [nht_res[c].astype(np.float32).reshape(H, BC).T for c in range(N_CORES)],
        axis=0,
    )
    new_cell = np.concatenate(
        [nclt_res[c].astype(np.float32).reshape(H, BC).T for c in range(N_CORES)],
        axis=0,
    )
    return new_hidden, new_cell



# revision 21
# speedup vs baseline: 1.1077x; 1.1077x over previous
"""Trainium2 Bass kernel for a fused CustomLSTMCell.

Math (reference):
    combined = concat([input, hidden], axis=1)            # [B, D], D = 2048
    gates    = combined @ concat([W_i,W_f,W_o,W_g], 1) + b  # [B, 4H]
    i, f, o, g = split(gates, 4, axis=1)
    new_cell   = sigmoid(f) * cell_state + sigmoid(i) * tanh(g)
    new_hidden = sigmoid(o) * tanh(new_cell)

Strategy:
  - Data-parallel over batch: 8 cores x 1024 rows each. No collectives.
  - Transposed W-stationary layout: gate columns (H) live on PSUM partitions,
    batch on the free dim. gates^T = W^T @ combined^T per 128-column h-block.
    Host prepares combined^T (bf16), W packed per h-block, cell_state^T.
  - The per-gate bias is then per-partition, so it folds into the ACT
    activation (out = sigmoid/tanh(psum + bias)) -- no DVE bias adds.
  - The cell/hidden elementwise chain runs on DVE in bf16 (2x rate). fp32
    drain math measured ~250us of DVE occupancy and periodically stalled the
    PE by delaying PSUM recycling; bf16 + ACT-bias cuts it to ~30us.
  - W streams as 8 x 2MB h-blocks through a 4-deep buffer ring, per-k DMA
    slices interleaved with the combined^T loads, so the PE never starves
    and the For_i back-edge overlaps.
"""

import sys

if "/opt/trn_rl_repo" not in sys.path:
    sys.path.insert(0, "/opt/trn_rl_repo")

import ml_dtypes
import numpy as np

import concourse.bass as bass
import concourse.mybir as mybir
import concourse.tile as tile
from concourse import bacc
from concourse.bass_utils import run_bass_kernel_spmd

N_CORES = 8
B = 8192
IN_SIZE = 1024
H = 1024
D = IN_SIZE + H          # 2048 contraction dim
G4 = 4 * H               # 4096 gate columns
BC = B // N_CORES        # 1024 batch rows per core
P = 128                  # partitions
KT = D // P              # 16 k-tiles
JT = H // P              # 8 h-blocks
NB = 512                 # batch columns per matmul (moving free dim)
BBLK = BC // NB          # 2 batch blocks

_NC_CACHE = {}


def _build(iters: int = 1) -> bass.Bass:
    nc = bacc.Bacc("TRN2", target_bir_lowering=False, debug=False)

    at = nc.dram_tensor("at", [D, BC], mybir.dt.bfloat16, kind="ExternalInput")
    wj = nc.dram_tensor("wj", [JT, P, KT * 512], mybir.dt.bfloat16, kind="ExternalInput")
    bv = nc.dram_tensor("bv", [P, 4 * JT], mybir.dt.float32, kind="ExternalInput")
    cst = nc.dram_tensor("cst", [JT, P, BC], mybir.dt.bfloat16, kind="ExternalInput")
    nht = nc.dram_tensor("nht", [JT, P, BC], mybir.dt.bfloat16, kind="ExternalOutput")
    nclt = nc.dram_tensor("nclt", [JT, P, BC], mybir.dt.bfloat16, kind="ExternalOutput")

    at_r = at.rearrange("(ko ki) b -> ki ko b", ki=P)   # [128, KT, BC]

    AF = mybir.ActivationFunctionType

    from contextlib import nullcontext

    with tile.TileContext(nc) as tc:
        with (
            tc.tile_pool(name="resident", bufs=1) as rpool,
            tc.tile_pool(name="wstream", bufs=4) as wspool,
            tc.tile_pool(name="work", bufs=2) as wpool,
            tc.tile_pool(name="psum", bufs=2, space="PSUM") as ppool,
        ):
            with (tc.For_i(0, iters, 1) if iters > 1 else nullcontext()):
                bv_sb = rpool.tile([P, 4 * JT], mybir.dt.float32, tag="bv")
                nc.sync.dma_start(out=bv_sb[:], in_=bv[:])
                at_sb = rpool.tile([P, KT, BC], mybir.dt.bfloat16, tag="at")
                for j in range(JT):
                    # stream this h-block of W (2 MB), interleaved per-k with
                    # the combined^T loads on the first block
                    wj_sb = wspool.tile([P, KT, 512], mybir.dt.bfloat16, tag="wj", name="wj_sb")
                    for k in range(KT):
                        nc.sync.dma_start(
                            out=wj_sb[:, k, :], in_=wj[j, :, k * 512 : (k + 1) * 512]
                        )
                        if j == 0:
                            nc.sync.dma_start(out=at_sb[:, k, :], in_=at_r[:, k, :])
                    for b in range(BBLK):
                        # prefetch the cell-state block a full group early
                        cl = wpool.tile([P, NB], mybir.dt.bfloat16, tag="cl")
                        nc.sync.dma_start(
                            out=cl[:], in_=cst[j, :, b * NB : (b + 1) * NB]
                        )
                        ps = [
                            ppool.tile([P, NB], mybir.dt.float32, tag=f"ps{q}", name=f"ps{q}")
                            for q in range(4)
                        ]
                        for k in range(KT):
                            mov = at_sb[:, k, b * NB : (b + 1) * NB]
                            for q in range(4):
                                nc.tensor.matmul(
                                    ps[q][:],
                                    lhsT=wj_sb[:, k, q * P : (q + 1) * P],
                                    rhs=mov,
                                    start=(k == 0),
                                    stop=(k == KT - 1),
                                )
                        # gate activations with fused per-partition bias,
                        # PSUM -> SBUF bf16
                        gt = [
                            wpool.tile([P, NB], mybir.dt.bfloat16, tag=f"gt{q}", name=f"gt{q}")
                            for q in range(4)
                        ]
                        for q in range(4):
                            nc.scalar.activation(
                                gt[q][:],
                                ps[q][:],
                                AF.Tanh if q == 3 else AF.Sigmoid,
                                bias=bv_sb[:, q * JT + j : q * JT + j + 1],
                            )
                        si, sf, so, sg = gt
                        # new_cell = sigmoid(f)*c + sigmoid(i)*tanh(g)  -> sf
                        nc.vector.tensor_mul(out=sf[:], in0=sf[:], in1=cl[:])
                        nc.vector.tensor_mul(out=si[:], in0=si[:], in1=sg[:])
                        nc.vector.tensor_add(out=sf[:], in0=sf[:], in1=si[:])
                        # new_hidden = sigmoid(o)*tanh(new_cell)        -> so
                        nc.scalar.activation(sg[:], sf[:], AF.Tanh)
                        nc.vector.tensor_mul(out=so[:], in0=so[:], in1=sg[:])
                        nc.sync.dma_start(
                            out=nclt[j, :, b * NB : (b + 1) * NB], in_=sf[:]
                        )
                        nc.sync.dma_start(
                            out=nht[j, :, b * NB : (b + 1) * NB], in_=so[:]
                        )
    nc.finalize()
    return nc


def get_nc(iters: int = 1) -> bass.Bass:
    if iters not in _NC_CACHE:
        _NC_CACHE[iters] = _build(iters)
    return _NC_CACHE[iters]


def make_in_maps(input, hidden, cell_state, W_i, b_i, W_f, b_f, W_o, b_o, W_g, b_g):
    comb = np.concatenate(
        [np.asarray(input, np.float32), np.asarray(hidden, np.float32)], axis=1
    )  # [B, D]
    W4 = np.concatenate(
        [np.asarray(W_i), np.asarray(W_f), np.asarray(W_o), np.asarray(W_g)], axis=1
    ).astype(np.float32)  # [D, 4H]
    b = np.concatenate(
        [np.asarray(b_i), np.asarray(b_f), np.asarray(b_o), np.asarray(b_g)]
    ).astype(np.float32)  # [4H]

    at_full = comb.T.astype(ml_dtypes.bfloat16)  # [D, B]
    # W packed per h-block j: [128 ki, KT, (q, col)] so per-k DMA slices are
    # contiguous and the stationary operand for (k, q) is wj[:, k, q*128:...]
    wj = np.empty((JT, P, KT * 512), dtype=ml_dtypes.bfloat16)
    for j in range(JT):
        blk = np.concatenate(
            [W4[:, q * H + j * P : q * H + (j + 1) * P] for q in range(4)], axis=1
        )  # [D, 512], col = q*128 + c
        wj[j] = (
            blk.astype(ml_dtypes.bfloat16)
            .reshape(KT, P, 512)
            .transpose(1, 0, 2)
            .reshape(P, KT * 512)
        )
    bv = np.ascontiguousarray(
        b.reshape(4, JT, P).transpose(2, 0, 1).reshape(P, 4 * JT)
    )
    cs = np.asarray(cell_state, np.float32)

    in_maps = []
    for c in range(N_CORES):
        sl = slice(c * BC, (c + 1) * BC)
        cst = np.ascontiguousarray(
            cs[sl].T.reshape(JT, P, BC).astype(ml_dtypes.bfloat16)
        )
        in_maps.append(
            {
                "at": np.ascontiguousarray(at_full[:, sl]),
                "wj": wj,
                "bv": bv,
                "cst": cst,
            }
        )
    return in_maps


def kernel(input, hidden, cell_state, W_i, b_i, W_f, b_f, W_o, b_o, W_g, b_g):
    in_maps = make_in_maps(
        input, hidden, cell_state, W_i, b_i, W_f, b_f, W_o, b_o, W_g, b_g
    )
    nc = get_nc(1)

    def run_once():
        # the device occasionally wedges transiently (NRT_EXEC_UNIT_
        # UNRECOVERABLE observed once today); retry before giving up
        import time as _time

        for attempt in range(3):
            try:
                res = run_bass_kernel_spmd(nc, in_maps, core_ids=list(range(N_CORES)))
                return (
                    [np.asarray(res.results[c]["nht"]).copy() for c in range(N_CORES)],
                    [np.asarray(res.results[c]["nclt"]).copy() for c in range(N_CORES)],
                )
            except Exception:
                if attempt == 2:
                    raise
                _time.sleep(10)

    # The device very occasionally corrupts an execution (observed ~once in
    # 14 runs on this host). The kernel is deterministic, so run twice and
    # accept only agreeing outputs; retry once more on mismatch.
    prev = run_once()
    for _ in range(2):
        cur = run_once()
        if all(
            np.array_equal(a, b) for a, b in zip(prev[0] + prev[1], cur[0] + cur[1])
        ):
            break
        prev = cur
    nht_res, nclt_res = prev
    new_hidden = np.concatenate(
        [nht_res[c].astype(np.float32).reshape(H, BC).T for c in range(N_CORES)],
        axis=0,
    )
    new_cell = np.concatenate(
        [nclt_res[c].astype(np.float32).reshape(H, BC).T for c in range(N_CORES)],
        axis=0,
    )
    return new_hidden, new_cell

